# revision 35
# baseline (speedup 1.0000x reference)
"""Trainium2 Bass kernel: 16-head attention (SEQ=4096, D_MODEL=1024, D_K=64).

Sharding: tensor-parallel over heads. 2 heads per core x 8 cores.
W_O is row-sharded; each core returns a partial [S, D] output projection,
summed on the host (the all-reduce of the output projection).

Per-core dataflow (score matmuls fp32r = FP22-truncated full-rate):
  qaug/kaug [65, S] per head via projections on transposed inputs; fp16
    copies q16/k16 feed the natural max pass.
  v natural [S, 64+1] fp16 per head (ones column yields softmax row sums).
  Natural pass (scores [q,s], fp16): DVE reduce_max per [128,512] unit into
    mxs, final reduce -> m2, transposed via a tiny -I matmul into qaug row 64.
    Streams inside the previous q-tile's main loop on a dedicated 2-buffer
    PSUM pool so reduce reads never stall the PE queue.
  Fine pass (transposed scores [s,q]): K=65 f32r matmul gives
    scores^T - max(q) on a 2-buffer [128,1024] PSUM pool; per-head ScalarE
    exp -> E fp16; [v|1] @ E accumulates att@V + row sums with a 2-chunk
    skew so PE never waits on the exp.
  Normalize via reciprocal + ones-replication matmul; W_O blocks of tile
    t-1 interleave into tile t's loop (c%8==1) on the fine pool.
"""

import os
import sys

import numpy as np

for _p in (
    "/root/.axon_site",
    "/root/.axon_site/_ro/trn_rl_repo",
    "/root/.axon_site/_ro/pypackages",
    "/opt/trn_rl_repo",
    "/opt/pypackages",
):
    if os.path.isdir(_p) and _p not in sys.path:
        sys.path.append(_p)

D = 1024
NHEADS = 16
DK = 64
NCORES = 8
S_FULL = 4096

_cache = {}
LAST_RESULT = None  # BassKernelResults of the most recent run (for test harness)


def _build(S):
    import concourse.bass as bass  # noqa: F401
    import concourse.tile as tile
    from concourse import bacc, mybir
    from concourse.masks import make_identity
    from contextlib import ExitStack

    f32 = mybir.dt.float32
    f32r = mybir.dt.float32r
    fp16 = mybir.dt.float16
    X = mybir.AxisListType.X
    Exp = mybir.ActivationFunctionType.Exp

    NT = S // 512   # 512-wide q tiles
    NCH = S // 128  # 128-wide s chunks
    ND = D // 128   # contraction chunks
    NSH = S // 512  # 512-wide s tiles

    dbg_on = bool(int(os.environ.get("KDBG", "0")))
    nc = bacc.Bacc(
        "TRN2",
        target_bir_lowering=False,
        debug=False,
        num_devices=NCORES,
    )
    qt = nc.dram_tensor("qt", [D, S], f32r, kind="ExternalInput")
    kt = nc.dram_tensor("kt", [D, S], f32r, kind="ExternalInput")
    vt = nc.dram_tensor("vt", [D, S], fp16, kind="ExternalInput")
    wq = nc.dram_tensor("wq", [D, 128], f32r, kind="ExternalInput")
    wk = nc.dram_tensor("wk", [D, 128], f32r, kind="ExternalInput")
    wv = nc.dram_tensor("wv", [D, 128], fp16, kind="ExternalInput")
    wo = nc.dram_tensor("wo", [128, D], f32r, kind="ExternalInput")
    out = nc.dram_tensor("out", [S, D], fp16, kind="ExternalOutput")
    dbg = None
    if dbg_on:
        dbg = {
            "d_qaug0": nc.dram_tensor("d_qaug0", [65, S], f32, kind="ExternalOutput"),
            "d_kaug0": nc.dram_tensor("d_kaug0", [65, S], f32, kind="ExternalOutput"),
            "d_v": nc.dram_tensor("d_v", [128, (S // 128) * 2 * 65], fp16,
                                  kind="ExternalOutput"),
            "d_concat": nc.dram_tensor("d_concat", [128, S], f32,
                                       kind="ExternalOutput"),
        }

    with tile.TileContext(nc) as tc, ExitStack() as ctx:
        consts = ctx.enter_context(tc.tile_pool(name="consts", bufs=1))
        big = ctx.enter_context(tc.tile_pool(name="big", bufs=1))
        ldpool = ctx.enter_context(tc.tile_pool(name="ld", bufs=2))
        epool = ctx.enter_context(tc.tile_pool(name="e", bufs=3))
        smalls = ctx.enter_context(tc.tile_pool(name="smalls", bufs=2))
        outp = ctx.enter_context(tc.tile_pool(name="outp", bufs=2))
        ps_f = ctx.enter_context(tc.tile_pool(name="ps_f", bufs=2, space="PSUM"))
        ps_n = ctx.enter_context(tc.tile_pool(name="ps_n", bufs=2, space="PSUM"))
        ps_av = ctx.enter_context(tc.tile_pool(name="ps_av", bufs=1, space="PSUM"))

        def pfine():
            return ps_f.tile([128, 1024], f32, tag="pf", name="pf")

        def pnat():
            return ps_n.tile([128, 512], f32, tag="pn", name="pn")

        # constants
        ident_f = consts.tile([128, 128], f32)
        make_identity(nc, ident_f)
        identp = consts.tile([128, 128], fp16)
        nc.scalar.copy(identp[:], ident_f[:])
        ones64 = consts.tile([1, 64], f32r)
        nc.vector.memset(ones64[:].bitcast(f32), 1.0)

        # weights
        wq_sb = consts.tile([128, ND, 128], f32r)
        nc.sync.dma_start(wq_sb[:], wq.rearrange("(o p) f -> p o f", p=128))
        wk_sb = consts.tile([128, ND, 128], f32r)
        nc.sync.dma_start(wk_sb[:], wk.rearrange("(o p) f -> p o f", p=128))
        wv_sb = consts.tile([128, ND, 128], fp16)
        nc.sync.dma_start(wv_sb[:], wv.rearrange("(o p) f -> p o f", p=128))
        wo_sb = consts.tile([128, D], f32r)
        nc.sync.dma_start(wo_sb[:], wo[:])

        # big SBUF tensors
        qaug = [big.tile([65, S], f32r, tag=f"qaug{h}", name=f"qaug{h}") for h in range(2)]
        kaug = [big.tile([65, S], f32r, tag=f"kaug{h}", name=f"kaug{h}") for h in range(2)]
        v_sb = big.tile([128, NCH, 2, 65], fp16, tag="v", name="v_sb")
        q16 = big.tile([128, S], fp16, tag="q16", name="q16")
        k16 = big.tile([128, S], fp16, tag="k16", name="k16")
        concat = big.tile([128, S], f32r, tag="concat", name="concat")
        for h in range(2):
            nc.vector.memset(kaug[h][64:65, :].bitcast(f32), 1.0)
        nc.gpsimd.memset(v_sb[:, :, :, 64:65], 1.0)

        # ---- per-tile projection helpers
        def proj_qk_tile(src, wsb, dstA, dstB, dst16, t):
            lt = ldpool.tile([128, ND, 512], f32r, tag="ld", name="ld", bufs=2)
            nc.sync.dma_start(
                lt[:], src[:, t * 512:(t + 1) * 512].rearrange("(o p) s -> p o s", p=128))
            ps = pnat()
            for d in range(ND):
                nc.tensor.matmul(ps[:], wsb[:, d, :], lt[:, d, :],
                                 start=(d == 0), stop=(d == ND - 1))
            nc.scalar.copy(dstA[0:64, t * 512:(t + 1) * 512], ps[0:64, :])
            nc.scalar.copy(dstB[0:64, t * 512:(t + 1) * 512], ps[64:128, :])
            nc.scalar.copy(dst16[:, t * 512:(t + 1) * 512], ps[:])

        def proj_v_tile(t):
            # sc-major: one accumulation group at a time per PSUM bank — a
            # group's start=True clears has_written for the whole bank, so
            # groups must not interleave within a bank.
            vts = ldpool.tile([128, ND, 512], fp16, tag="vld", name="vld", bufs=2)
            nc.sync.dma_start(
                vts[:], vt[:, t * 512:(t + 1) * 512].rearrange("(o p) s -> p o s", p=128))
            ps = pnat()
            for sc in range(4):
                for d in range(ND):
                    nc.tensor.matmul(ps[:, sc * 128:(sc + 1) * 128],
                                     vts[:, d, sc * 128:(sc + 1) * 128],
                                     wv_sb[:, d, :],
                                     start=(d == 0), stop=(d == ND - 1),
                                     skip_group_check=True)
            for sc in range(4):
                c = t * 4 + sc
                nc.scalar.copy(
                    v_sb[:, c, :, 0:64],
                    ps[:, sc * 128:(sc + 1) * 128].rearrange("p (h f) -> p h f", h=2))

        # ---- natural max pass pieces
        mxs_all = {}

        def nat_unit(b, sh, h):
            qsl = slice(b * 128, (b + 1) * 128)
            ssl = slice(sh * 512, (sh + 1) * 512)
            hp = slice(h * 64, (h + 1) * 64)
            psn = pnat()
            nc.tensor.matmul(psn[:], q16[hp, qsl], k16[hp, ssl],
                             start=True, stop=True)
            if (b, h) not in mxs_all:
                mxs_all[(b, h)] = smalls.tile(
                    [128, NSH], f32, tag=f"mx{b % 4}_{h}", name=f"mx{b % 4}_{h}")
            nc.vector.reduce_max(mxs_all[(b, h)][:, sh:sh + 1], psn[:], axis=X)

        def nat_finish(b):
            qsl = slice(b * 128, (b + 1) * 128)
            m2 = smalls.tile([128, 2], fp16, tag="m2", name="m2")
            for h in range(2):
                nc.vector.reduce_max(m2[:, h:h + 1], mxs_all.pop((b, h))[:],
                                     axis=X, negate=True)
            for h in range(2):
                psmt = pnat()
                nc.tensor.matmul(psmt[0:1, 0:128], m2[:, h:h + 1], identp[:],
                                 start=True, stop=True)
                nc.vector.tensor_copy(qaug[h][64:65, qsl], psmt[0:1, 0:128])

        # ---- phase A: q-tile-0 proj first, then kt proj with the natural
        # pass streamed into each tile's DMA window, then v proj, qt tile 1.
        proj_qk_tile(qt, wq_sb, qaug[0], qaug[1], q16, 0)
        for sh in range(NSH):
            proj_qk_tile(kt, wk_sb, kaug[0], kaug[1], k16, sh)
            for b in range(4):
                for h in range(2):
                    nat_unit(b, sh, h)
        for sh in range(NSH):
            proj_v_tile(sh)
        proj_qk_tile(qt, wq_sb, qaug[0], qaug[1], q16, 1)
        for b in range(4):
            nat_finish(b)

        # ---- main loop over q tiles
        wo_queue = []  # deferred W_O block indices (qb values)

        def wo_block(qb, copy_eng=None):
            pso = pfine()
            for n in range(2):
                nc.tensor.matmul(pso[:, n * 512:(n + 1) * 512],
                                 concat[:, qb * 128:(qb + 1) * 128],
                                 wo_sb[:, n * 512:(n + 1) * 512],
                                 start=True, stop=True)
            ot = outp.tile([128, 1024], fp16, tag="ot", name="ot")
            (copy_eng or nc.scalar.copy)(ot[:], pso[:])
            nc.sync.dma_start(out[qb * 128:(qb + 1) * 128, :], ot[:])

        for t in range(NT):
            tsl = slice(t * 512, (t + 1) * 512)
            psAs = [ps_av.tile([65, 512], f32, tag=f"pav{h}", name=f"pav{h}")
                    for h in range(2)]
            es = {}
            for c in range(NCH):
                psf = pfine()
                for h in range(2):
                    nc.tensor.matmul(psf[:, h * 512:(h + 1) * 512],
                                     kaug[h][:, c * 128:(c + 1) * 128],
                                     qaug[h][:, tsl],
                                     start=True, stop=True)
                if c >= 2:
                    eprev = es.pop(c - 2)
                    for h in range(2):
                        nc.tensor.matmul(psAs[h][:], v_sb[:, c - 2, h, :],
                                         eprev[:, h * 512:(h + 1) * 512],
                                         start=(c - 2 == 0), stop=False,
                                         skip_group_check=True)
                e = epool.tile([128, 1024], fp16, tag="e", name="e")
                nc.scalar.activation(e[:], psf[:], Exp)
                es[c] = e
                # W_O of previous tile, spread through this loop
                if c % 8 == 1 and wo_queue:
                    wo_block(wo_queue.pop(0))
                # stream next q-tile's natural pass; finish one step late so
                # the psmt matmul never blocks the in-order PE queue on DVE
                if t + 1 < NT:
                    b, sh = divmod(c, NSH)
                    for h in range(2):
                        nat_unit((t + 1) * 4 + b, sh, h)
                    if c % NSH == 0 and c >= NSH:
                        nat_finish((t + 1) * 4 + c // NSH - 1)
            # tail AV chunks
            for cc in (NCH - 2, NCH - 1):
                eprev = es.pop(cc)
                for h in range(2):
                    nc.tensor.matmul(psAs[h][:], v_sb[:, cc, h, :],
                                     eprev[:, h * 512:(h + 1) * 512],
                                     start=False, stop=(cc == NCH - 1),
                                     skip_group_check=True)
            if t + 1 < NT:
                nat_finish((t + 1) * 4 + 3)

            # normalize: concat[h] = att@V * (1/rowsum), transposed layout
            for h in range(2):
                psA = psAs[h]
                sums = smalls.tile([1, 512], f32, tag="sums", name="sums", bufs=1)
                nc.vector.tensor_copy(sums[:], psA[64:65, :])
                recf = smalls.tile([1, 512], f32, tag="recf", name="recf", bufs=1)
                nc.vector.reciprocal_approx_fast(recf[:], sums[:])
                rec = smalls.tile([1, 512], f32r, tag="rec", name="rec", bufs=1)
                nc.vector.tensor_copy(rec[:], recf[:])
                psr = pfine()
                nc.tensor.matmul(psr[0:64, 0:512], ones64[:], rec[:],
                                 start=True, stop=True)
                reps = smalls.tile([64, 512], f32, tag="reps", name="reps", bufs=1)
                nc.scalar.copy(reps[:], psr[0:64, 0:512])
                nc.vector.tensor_mul(concat[h * 64:(h + 1) * 64, tsl],
                                     psA[0:64, :], reps[:])

            # qt proj for tile t+2 (needed by nat pass streaming during t+1)
            if t + 2 < NT:
                proj_qk_tile(qt, wq_sb, qaug[0], qaug[1], q16, t + 2)

            wo_queue.extend(t * 4 + b for b in range(4))
            if t == NT - 1:
                for i in range(len(wo_queue)):
                    wo_block(wo_queue.pop(0))

        if dbg is not None:
            nc.sync.dma_start(dbg["d_qaug0"][:], qaug[0][:].bitcast(f32))
            nc.sync.dma_start(dbg["d_kaug0"][:], kaug[0][:].bitcast(f32))
            nc.sync.dma_start(dbg["d_v"][:], v_sb[:].rearrange("p a b c -> p (a b c)"))
            nc.sync.dma_start(dbg["d_concat"][:], concat[:].bitcast(f32))

    nc.compile()
    return nc


def _prep_inputs(Q, K, V, W_Q, W_K, W_V, W_O):
    Q = np.ascontiguousarray(np.asarray(Q, dtype=np.float32))
    K = np.ascontiguousarray(np.asarray(K, dtype=np.float32))
    V = np.ascontiguousarray(np.asarray(V, dtype=np.float32))
    W_Q = np.asarray(W_Q, dtype=np.float32)
    W_K = np.asarray(W_K, dtype=np.float32)
    W_V = np.asarray(W_V, dtype=np.float32)
    W_O = np.asarray(W_O, dtype=np.float32)

    QT = np.ascontiguousarray(Q.T)
    KT = np.ascontiguousarray(K.T)
    VT = np.ascontiguousarray(V.T.astype(np.float16))
    scale = np.float32(0.125)  # 1/sqrt(64), exact power of two

    in_maps = []
    for c in range(NCORES):
        hA, hB = 2 * c, 2 * c + 1
        in_maps.append({
            "qt": QT,
            "kt": KT,
            "vt": VT,
            "wq": np.ascontiguousarray(np.concatenate([W_Q[hA], W_Q[hB]], axis=1)),
            "wk": np.ascontiguousarray(
                np.concatenate([W_K[hA] * scale, W_K[hB] * scale], axis=1)),
            "wv": np.ascontiguousarray(
                np.concatenate([W_V[hA], W_V[hB]], axis=1).astype(np.float16)),
            "wo": np.ascontiguousarray(W_O[c * 128:(c + 1) * 128, :]),
        })
    return in_maps


def kernel(Q, K, V, W_Q, W_K, W_V, W_O):
    global LAST_RESULT
    from concourse.bass_utils import run_bass_kernel_spmd

    S = np.asarray(Q).shape[0]
    nc = _cache.get(S)
    if nc is None:
        nc = _build(S)
        _cache[S] = nc

    in_maps = _prep_inputs(Q, K, V, W_Q, W_K, W_V, W_O)
    res = run_bass_kernel_spmd(nc, in_maps, list(range(NCORES)))
    LAST_RESULT = res
    parts = np.stack(
        [res.results[i]["out"].astype(np.float32) for i in range(NCORES)])
    return parts.sum(axis=0, dtype=np.float32)


# revision 37
# speedup vs baseline: 1.1360x; 1.1360x over previous
"""Trainium2 Bass kernel: 16-head attention (SEQ=4096, D_MODEL=1024, D_K=64).

Sharding: tensor-parallel over heads. 2 heads per core x 8 cores.
W_O is row-sharded; each core returns a partial [S, D] output projection,
summed on the host (the all-reduce of the output projection).

Per-core dataflow (score matmuls fp32r = FP22-truncated full-rate):
  qaug/kaug [65, S] per head via projections on transposed inputs; fp16
    copies q16/k16 feed the natural max pass.
  v natural [S, 64+1] fp16 per head (ones column yields softmax row sums).
  Natural pass (scores [q,s], fp16): DVE reduce_max per [128,512] unit into
    mxs, final reduce -> m2, transposed via a tiny -I matmul into qaug row 64.
    Streams inside the previous q-tile's main loop on a dedicated 2-buffer
    PSUM pool so reduce reads never stall the PE queue.
  Fine pass (transposed scores [s,q]): K=65 f32r matmul gives
    scores^T - max(q) on a 2-buffer [128,1024] PSUM pool; per-head ScalarE
    exp -> E fp16; [v|1] @ E accumulates att@V + row sums with a 2-chunk
    skew so PE never waits on the exp.
  Normalize via reciprocal + ones-replication matmul; W_O blocks of tile
    t-1 interleave into tile t's loop (c%8==1) on the fine pool.
"""

import os
import sys

import numpy as np

for _p in (
    "/root/.axon_site",
    "/root/.axon_site/_ro/trn_rl_repo",
    "/root/.axon_site/_ro/pypackages",
    "/opt/trn_rl_repo",
    "/opt/pypackages",
):
    if os.path.isdir(_p) and _p not in sys.path:
        sys.path.append(_p)

D = 1024
NHEADS = 16
DK = 64
NCORES = 8
S_FULL = 4096

_cache = {}
LAST_RESULT = None  # BassKernelResults of the most recent run (for test harness)


def _build(S):
    import concourse.bass as bass  # noqa: F401
    import concourse.tile as tile
    from concourse import bacc, mybir
    from concourse.masks import make_identity
    from contextlib import ExitStack

    f32 = mybir.dt.float32
    f32r = mybir.dt.float32r
    fp16 = mybir.dt.float16
    X = mybir.AxisListType.X
    Exp = mybir.ActivationFunctionType.Exp

    NT = S // 512   # 512-wide q tiles
    NCH = S // 128  # 128-wide s chunks
    ND = D // 128   # contraction chunks
    NSH = S // 512  # 512-wide s tiles

    dbg_on = bool(int(os.environ.get("KDBG", "0")))
    nc = bacc.Bacc(
        "TRN2",
        target_bir_lowering=False,
        debug=False,
        num_devices=NCORES,
    )
    qt = nc.dram_tensor("qt", [D, S], f32r, kind="ExternalInput")
    kt = nc.dram_tensor("kt", [D, S], f32r, kind="ExternalInput")
    vt = nc.dram_tensor("vt", [D, S], fp16, kind="ExternalInput")
    wq = nc.dram_tensor("wq", [D, 128], f32r, kind="ExternalInput")
    wk = nc.dram_tensor("wk", [D, 128], f32r, kind="ExternalInput")
    wv = nc.dram_tensor("wv", [D, 128], fp16, kind="ExternalInput")
    wo = nc.dram_tensor("wo", [128, D], f32r, kind="ExternalInput")
    out = nc.dram_tensor("out", [S, D], fp16, kind="ExternalOutput")
    dbg = None
    if dbg_on:
        dbg = {
            "d_qaug0": nc.dram_tensor("d_qaug0", [65, S], f32, kind="ExternalOutput"),
            "d_kaug0": nc.dram_tensor("d_kaug0", [65, S], f32, kind="ExternalOutput"),
            "d_v": nc.dram_tensor("d_v", [128, (S // 128) * 2 * 65], fp16,
                                  kind="ExternalOutput"),
            "d_concat": nc.dram_tensor("d_concat", [128, S], f32,
                                       kind="ExternalOutput"),
        }

    with tile.TileContext(nc) as tc, ExitStack() as ctx:
        consts = ctx.enter_context(tc.tile_pool(name="consts", bufs=1))
        big = ctx.enter_context(tc.tile_pool(name="big", bufs=1))
        ldpool = ctx.enter_context(tc.tile_pool(name="ld", bufs=2))
        epool = ctx.enter_context(tc.tile_pool(name="e", bufs=3))
        smalls = ctx.enter_context(tc.tile_pool(name="smalls", bufs=2))
        outp = ctx.enter_context(tc.tile_pool(name="outp", bufs=2))
        ps_f = ctx.enter_context(tc.tile_pool(name="ps_f", bufs=2, space="PSUM"))
        ps_n = ctx.enter_context(tc.tile_pool(name="ps_n", bufs=2, space="PSUM"))
        ps_av = ctx.enter_context(tc.tile_pool(name="ps_av", bufs=1, space="PSUM"))

        def pfine():
            return ps_f.tile([128, 1024], f32, tag="pf", name="pf")

        def pnat():
            return ps_n.tile([128, 512], f32, tag="pn", name="pn")

        # constants
        ident_f = consts.tile([128, 128], f32)
        make_identity(nc, ident_f)
        identp = consts.tile([128, 128], fp16)
        nc.scalar.copy(identp[:], ident_f[:])
        ones64 = consts.tile([1, 64], f32r)
        nc.vector.memset(ones64[:].bitcast(f32), 1.0)

        # weights
        wq_sb = consts.tile([128, ND, 128], f32r)
        nc.sync.dma_start(wq_sb[:], wq.rearrange("(o p) f -> p o f", p=128))
        wk_sb = consts.tile([128, ND, 128], f32r)
        nc.sync.dma_start(wk_sb[:], wk.rearrange("(o p) f -> p o f", p=128))
        wv_sb = consts.tile([128, ND, 128], fp16)
        nc.sync.dma_start(wv_sb[:], wv.rearrange("(o p) f -> p o f", p=128))
        wo_sb = consts.tile([128, D], f32r)
        nc.sync.dma_start(wo_sb[:], wo[:])

        # big SBUF tensors
        qaug = [big.tile([65, S], f32r, tag=f"qaug{h}", name=f"qaug{h}") for h in range(2)]
        kaug = [big.tile([65, S], f32r, tag=f"kaug{h}", name=f"kaug{h}") for h in range(2)]
        v_sb = big.tile([128, NCH, 2, 65], fp16, tag="v", name="v_sb")
        q16 = big.tile([128, S], fp16, tag="q16", name="q16")
        k16 = big.tile([128, S], fp16, tag="k16", name="k16")
        concat = big.tile([128, S], f32r, tag="concat", name="concat")
        for h in range(2):
            nc.vector.memset(kaug[h][64:65, :].bitcast(f32), 1.0)
        nc.gpsimd.memset(v_sb[:, :, :, 64:65], 1.0)

        # ---- per-tile projection helpers
        def proj_qk_tile(src, wsb, dstA, dstB, dst16, t):
            lt = ldpool.tile([128, ND, 512], f32r, tag="ld", name="ld", bufs=2)
            nc.sync.dma_start(
                lt[:], src[:, t * 512:(t + 1) * 512].rearrange("(o p) s -> p o s", p=128))
            ps = pnat()
            for d in range(ND):
                nc.tensor.matmul(ps[:], wsb[:, d, :], lt[:, d, :],
                                 start=(d == 0), stop=(d == ND - 1))
            nc.scalar.copy(dstA[0:64, t * 512:(t + 1) * 512], ps[0:64, :])
            nc.scalar.copy(dstB[0:64, t * 512:(t + 1) * 512], ps[64:128, :])
            nc.scalar.copy(dst16[:, t * 512:(t + 1) * 512], ps[:])

        def proj_v_tile(t):
            # sc-major: one accumulation group at a time per PSUM bank — a
            # group's start=True clears has_written for the whole bank, so
            # groups must not interleave within a bank.
            vts = ldpool.tile([128, ND, 512], fp16, tag="vld", name="vld", bufs=2)
            nc.sync.dma_start(
                vts[:], vt[:, t * 512:(t + 1) * 512].rearrange("(o p) s -> p o s", p=128))
            ps = pnat()
            for sc in range(4):
                for d in range(ND):
                    nc.tensor.matmul(ps[:, sc * 128:(sc + 1) * 128],
                                     vts[:, d, sc * 128:(sc + 1) * 128],
                                     wv_sb[:, d, :],
                                     start=(d == 0), stop=(d == ND - 1),
                                     skip_group_check=True)
            for sc in range(4):
                c = t * 4 + sc
                nc.scalar.copy(
                    v_sb[:, c, :, 0:64],
                    ps[:, sc * 128:(sc + 1) * 128].rearrange("p (h f) -> p h f", h=2))

        # ---- natural max pass pieces
        mxs_all = {}

        def nat_unit(b, sh, h):
            qsl = slice(b * 128, (b + 1) * 128)
            ssl = slice(sh * 512, (sh + 1) * 512)
            hp = slice(h * 64, (h + 1) * 64)
            psn = pnat()
            nc.tensor.matmul(psn[:], q16[hp, qsl], k16[hp, ssl],
                             start=True, stop=True)
            if (b, h) not in mxs_all:
                mxs_all[(b, h)] = smalls.tile(
                    [128, NSH], f32, tag=f"mx{b % 4}_{h}", name=f"mx{b % 4}_{h}")
            nc.vector.reduce_max(mxs_all[(b, h)][:, sh:sh + 1], psn[:], axis=X)

        def nat_finish(b):
            qsl = slice(b * 128, (b + 1) * 128)
            m2 = smalls.tile([128, 2], fp16, tag="m2", name="m2")
            for h in range(2):
                nc.vector.reduce_max(m2[:, h:h + 1], mxs_all.pop((b, h))[:],
                                     axis=X, negate=True)
            for h in range(2):
                psmt = pnat()
                nc.tensor.matmul(psmt[0:1, 0:128], m2[:, h:h + 1], identp[:],
                                 start=True, stop=True)
                nc.vector.tensor_copy(qaug[h][64:65, qsl], psmt[0:1, 0:128])

        # ---- phase A: q-tile-0 proj first, then kt proj with the natural
        # pass streamed into each tile's DMA window, then v proj, qt tile 1.
        proj_qk_tile(qt, wq_sb, qaug[0], qaug[1], q16, 0)
        for sh in range(NSH):
            proj_qk_tile(kt, wk_sb, kaug[0], kaug[1], k16, sh)
            for b in range(4):
                for h in range(2):
                    nat_unit(b, sh, h)
        for sh in range(NSH):
            proj_v_tile(sh)
        proj_qk_tile(qt, wq_sb, qaug[0], qaug[1], q16, 1)
        for b in range(4):
            nat_finish(b)

        # ---- main loop over q tiles
        wo_queue = []  # deferred W_O block indices (qb values)

        def wo_block(qb, copy_eng=None):
            pso = pfine()
            for n in range(2):
                nc.tensor.matmul(pso[:, n * 512:(n + 1) * 512],
                                 concat[:, qb * 128:(qb + 1) * 128],
                                 wo_sb[:, n * 512:(n + 1) * 512],
                                 start=True, stop=True)
            ot = outp.tile([128, 1024], fp16, tag="ot", name="ot")
            (copy_eng or nc.scalar.copy)(ot[:], pso[:])
            nc.sync.dma_start(out[qb * 128:(qb + 1) * 128, :], ot[:])

        for t in range(NT):
            tsl = slice(t * 512, (t + 1) * 512)
            psAs = [ps_av.tile([65, 512], f32, tag=f"pav{h}", name=f"pav{h}")
                    for h in range(2)]
            es = {}
            for c in range(NCH):
                psf = pfine()
                for h in range(2):
                    nc.tensor.matmul(psf[:, h * 512:(h + 1) * 512],
                                     kaug[h][:, c * 128:(c + 1) * 128],
                                     qaug[h][:, tsl],
                                     start=True, stop=True)
                if c >= 2:
                    eprev = es.pop(c - 2)
                    for h in range(2):
                        nc.tensor.matmul(psAs[h][:], v_sb[:, c - 2, h, :],
                                         eprev[h][:],
                                         start=(c - 2 == 0), stop=False,
                                         skip_group_check=True)
                es[c] = []
                for h in range(2):
                    e = epool.tile([128, 512], fp16, tag=f"e{h}", name=f"e{h}")
                    nc.scalar.activation(e[:], psf[:, h * 512:(h + 1) * 512], Exp)
                    es[c].append(e)
                # W_O of previous tile, spread through this loop
                if c % 8 == 1 and wo_queue:
                    wo_block(wo_queue.pop(0))
                # stream next q-tile's natural pass; finish one step late so
                # the psmt matmul never blocks the in-order PE queue on DVE
                if t + 1 < NT:
                    b, sh = divmod(c, NSH)
                    for h in range(2):
                        nat_unit((t + 1) * 4 + b, sh, h)
                    if c % NSH == 0 and c >= NSH:
                        nat_finish((t + 1) * 4 + c // NSH - 1)
            # tail AV chunks
            for cc in (NCH - 2, NCH - 1):
                eprev = es.pop(cc)
                for h in range(2):
                    nc.tensor.matmul(psAs[h][:], v_sb[:, cc, h, :],
                                     eprev[h][:],
                                     start=False, stop=(cc == NCH - 1),
                                     skip_group_check=True)
            if t + 1 < NT:
                nat_finish((t + 1) * 4 + 3)

            # normalize: concat[h] = att@V * (1/rowsum), transposed layout
            for h in range(2):
                psA = psAs[h]
                sums = smalls.tile([1, 512], f32, tag="sums", name="sums", bufs=1)
                nc.vector.tensor_copy(sums[:], psA[64:65, :])
                recf = smalls.tile([1, 512], f32, tag="recf", name="recf", bufs=1)
                nc.vector.reciprocal_approx_fast(recf[:], sums[:])
                rec = smalls.tile([1, 512], f32r, tag="rec", name="rec", bufs=1)
                nc.vector.tensor_copy(rec[:], recf[:])
                psr = pfine()
                nc.tensor.matmul(psr[0:64, 0:512], ones64[:], rec[:],
                                 start=True, stop=True)
                reps = smalls.tile([64, 512], f32, tag="reps", name="reps", bufs=1)
                nc.scalar.copy(reps[:], psr[0:64, 0:512])
                nc.vector.tensor_mul(concat[h * 64:(h + 1) * 64, tsl],
                                     psA[0:64, :], reps[:])

            # qt proj for tile t+2 (needed by nat pass streaming during t+1)
            if t + 2 < NT:
                proj_qk_tile(qt, wq_sb, qaug[0], qaug[1], q16, t + 2)

            wo_queue.extend(t * 4 + b for b in range(4))
            if t == NT - 1:
                for i in range(len(wo_queue)):
                    wo_block(wo_queue.pop(0))

        if dbg is not None:
            nc.sync.dma_start(dbg["d_qaug0"][:], qaug[0][:].bitcast(f32))
            nc.sync.dma_start(dbg["d_kaug0"][:], kaug[0][:].bitcast(f32))
            nc.sync.dma_start(dbg["d_v"][:], v_sb[:].rearrange("p a b c -> p (a b c)"))
            nc.sync.dma_start(dbg["d_concat"][:], concat[:].bitcast(f32))

    nc.compile()
    return nc


def _prep_inputs(Q, K, V, W_Q, W_K, W_V, W_O):
    Q = np.ascontiguousarray(np.asarray(Q, dtype=np.float32))
    K = np.ascontiguousarray(np.asarray(K, dtype=np.float32))
    V = np.ascontiguousarray(np.asarray(V, dtype=np.float32))
    W_Q = np.asarray(W_Q, dtype=np.float32)
    W_K = np.asarray(W_K, dtype=np.float32)
    W_V = np.asarray(W_V, dtype=np.float32)
    W_O = np.asarray(W_O, dtype=np.float32)

    QT = np.ascontiguousarray(Q.T)
    KT = np.ascontiguousarray(K.T)
    VT = np.ascontiguousarray(V.T.astype(np.float16))
    scale = np.float32(0.125)  # 1/sqrt(64), exact power of two

    in_maps = []
    for c in range(NCORES):
        hA, hB = 2 * c, 2 * c + 1
        in_maps.append({
            "qt": QT,
            "kt": KT,
            "vt": VT,
            "wq": np.ascontiguousarray(np.concatenate([W_Q[hA], W_Q[hB]], axis=1)),
            "wk": np.ascontiguousarray(
                np.concatenate([W_K[hA] * scale, W_K[hB] * scale], axis=1)),
            "wv": np.ascontiguousarray(
                np.concatenate([W_V[hA], W_V[hB]], axis=1).astype(np.float16)),
            "wo": np.ascontiguousarray(W_O[c * 128:(c + 1) * 128, :]),
        })
    return in_maps


def kernel(Q, K, V, W_Q, W_K, W_V, W_O):
    global LAST_RESULT
    from concourse.bass_utils import run_bass_kernel_spmd

    S = np.asarray(Q).shape[0]
    nc = _cache.get(S)
    if nc is None:
        nc = _build(S)
        _cache[S] = nc

    in_maps = _prep_inputs(Q, K, V, W_Q, W_K, W_V, W_O)
    res = run_bass_kernel_spmd(nc, in_maps, list(range(NCORES)))
    LAST_RESULT = res
    parts = np.stack(
        [res.results[i]["out"].astype(np.float32) for i in range(NCORES)])
    return parts.sum(axis=0, dtype=np.float32)


# revision 39
# speedup vs baseline: 1.1411x; 1.0045x over previous
"""Trainium2 Bass kernel: 16-head attention (SEQ=4096, D_MODEL=1024, D_K=64).

Sharding: tensor-parallel over heads. 2 heads per core x 8 cores.
W_O is row-sharded; each core returns a partial [S, D] output projection,
summed on the host (the all-reduce of the output projection).

Per-core dataflow (score matmuls fp32r = FP22-truncated full-rate):
  qaug/kaug [65, S] per head via projections on transposed inputs; fp16
    copies q16/k16 feed the natural max pass.
  v natural [S, 64+1] fp16 per head (ones column yields softmax row sums).
  Natural pass (scores [q,s], fp16): DVE reduce_max per [128,512] unit into
    mxs, final reduce -> m2, transposed via a tiny -I matmul into qaug row 64.
    Streams inside the previous q-tile's main loop on a dedicated 2-buffer
    PSUM pool so reduce reads never stall the PE queue.
  Fine pass (transposed scores [s,q]): K=65 f32r matmul gives
    scores^T - max(q) on a 2-buffer [128,1024] PSUM pool; per-head ScalarE
    exp -> E fp16; [v|1] @ E accumulates att@V + row sums with a 2-chunk
    skew so PE never waits on the exp.
  Normalize via reciprocal + ones-replication matmul; W_O blocks of tile
    t-1 interleave into tile t's loop (c%8==1) on the fine pool.
"""

import os
import sys

import numpy as np

for _p in (
    "/root/.axon_site",
    "/root/.axon_site/_ro/trn_rl_repo",
    "/root/.axon_site/_ro/pypackages",
    "/opt/trn_rl_repo",
    "/opt/pypackages",
):
    if os.path.isdir(_p) and _p not in sys.path:
        sys.path.append(_p)

D = 1024
NHEADS = 16
DK = 64
NCORES = 8
S_FULL = 4096

_cache = {}
LAST_RESULT = None  # BassKernelResults of the most recent run (for test harness)


def _build(S):
    import concourse.bass as bass  # noqa: F401
    import concourse.tile as tile
    from concourse import bacc, mybir
    from concourse.masks import make_identity
    from contextlib import ExitStack

    f32 = mybir.dt.float32
    f32r = mybir.dt.float32r
    fp16 = mybir.dt.float16
    X = mybir.AxisListType.X
    Exp = mybir.ActivationFunctionType.Exp

    NT = S // 512   # 512-wide q tiles
    NCH = S // 128  # 128-wide s chunks
    ND = D // 128   # contraction chunks
    NSH = S // 512  # 512-wide s tiles

    dbg_on = bool(int(os.environ.get("KDBG", "0")))
    nc = bacc.Bacc(
        "TRN2",
        target_bir_lowering=False,
        debug=False,
        num_devices=NCORES,
    )
    qt = nc.dram_tensor("qt", [D, S], f32r, kind="ExternalInput")
    kt = nc.dram_tensor("kt", [D, S], f32r, kind="ExternalInput")
    vt = nc.dram_tensor("vt", [D, S], fp16, kind="ExternalInput")
    wq = nc.dram_tensor("wq", [D, 128], f32r, kind="ExternalInput")
    wk = nc.dram_tensor("wk", [D, 128], f32r, kind="ExternalInput")
    wv = nc.dram_tensor("wv", [D, 128], fp16, kind="ExternalInput")
    wo = nc.dram_tensor("wo", [128, D], f32r, kind="ExternalInput")
    out = nc.dram_tensor("out", [S, D], fp16, kind="ExternalOutput")
    dbg = None
    if dbg_on:
        dbg = {
            "d_qaug0": nc.dram_tensor("d_qaug0", [65, S], f32, kind="ExternalOutput"),
            "d_kaug0": nc.dram_tensor("d_kaug0", [65, S], f32, kind="ExternalOutput"),
            "d_v": nc.dram_tensor("d_v", [128, (S // 128) * 2 * 65], fp16,
                                  kind="ExternalOutput"),
            "d_concat": nc.dram_tensor("d_concat", [128, S], f32,
                                       kind="ExternalOutput"),
        }

    with tile.TileContext(nc) as tc, ExitStack() as ctx:
        consts = ctx.enter_context(tc.tile_pool(name="consts", bufs=1))
        big = ctx.enter_context(tc.tile_pool(name="big", bufs=1))
        ldpool = ctx.enter_context(tc.tile_pool(name="ld", bufs=2))
        epool = ctx.enter_context(tc.tile_pool(name="e", bufs=3))
        smalls = ctx.enter_context(tc.tile_pool(name="smalls", bufs=2))
        outp = ctx.enter_context(tc.tile_pool(name="outp", bufs=2))
        ps_f = ctx.enter_context(tc.tile_pool(name="ps_f", bufs=2, space="PSUM"))
        ps_n = ctx.enter_context(tc.tile_pool(name="ps_n", bufs=2, space="PSUM"))
        ps_av = ctx.enter_context(tc.tile_pool(name="ps_av", bufs=1, space="PSUM"))

        def pfine():
            return ps_f.tile([128, 1024], f32, tag="pf", name="pf")

        def pnat():
            return ps_n.tile([128, 512], f32, tag="pn", name="pn")

        # constants
        ident_f = consts.tile([128, 128], f32)
        make_identity(nc, ident_f)
        identp = consts.tile([128, 128], fp16)
        nc.scalar.copy(identp[:], ident_f[:])
        ones64 = consts.tile([1, 64], f32r)
        nc.vector.memset(ones64[:].bitcast(f32), 1.0)

        # weights
        wq_sb = consts.tile([128, ND, 128], f32r)
        nc.sync.dma_start(wq_sb[:], wq.rearrange("(o p) f -> p o f", p=128))
        wk_sb = consts.tile([128, ND, 128], f32r)
        nc.sync.dma_start(wk_sb[:], wk.rearrange("(o p) f -> p o f", p=128))
        wv_sb = consts.tile([128, ND, 128], fp16)
        nc.sync.dma_start(wv_sb[:], wv.rearrange("(o p) f -> p o f", p=128))
        wo_sb = consts.tile([128, D], f32r)
        nc.sync.dma_start(wo_sb[:], wo[:])

        # big SBUF tensors
        qaug = [big.tile([65, S], f32r, tag=f"qaug{h}", name=f"qaug{h}") for h in range(2)]
        kaug = [big.tile([65, S], f32r, tag=f"kaug{h}", name=f"kaug{h}") for h in range(2)]
        v_sb = big.tile([128, NCH, 2, 65], fp16, tag="v", name="v_sb")
        q16 = big.tile([128, S], fp16, tag="q16", name="q16")
        k16 = big.tile([128, S], fp16, tag="k16", name="k16")
        concat = big.tile([128, S], f32r, tag="concat", name="concat")
        for h in range(2):
            nc.vector.memset(kaug[h][64:65, :].bitcast(f32), 1.0)
        nc.gpsimd.memset(v_sb[:, :, :, 64:65], 1.0)

        # ---- per-tile projection helpers
        def proj_qk_tile(src, wsb, dstA, dstB, dst16, t):
            lt = ldpool.tile([128, ND, 512], f32r, tag="ld", name="ld", bufs=2)
            nc.sync.dma_start(
                lt[:], src[:, t * 512:(t + 1) * 512].rearrange("(o p) s -> p o s", p=128))
            ps = pnat()
            for d in range(ND):
                nc.tensor.matmul(ps[:], wsb[:, d, :], lt[:, d, :],
                                 start=(d == 0), stop=(d == ND - 1))
            nc.scalar.copy(dstA[0:64, t * 512:(t + 1) * 512], ps[0:64, :])
            nc.scalar.copy(dstB[0:64, t * 512:(t + 1) * 512], ps[64:128, :])
            nc.scalar.copy(dst16[:, t * 512:(t + 1) * 512], ps[:])

        def proj_v_tile(t):
            # sc-major: one accumulation group at a time per PSUM bank — a
            # group's start=True clears has_written for the whole bank, so
            # groups must not interleave within a bank.
            vts = ldpool.tile([128, ND, 512], fp16, tag="vld", name="vld", bufs=2)
            nc.sync.dma_start(
                vts[:], vt[:, t * 512:(t + 1) * 512].rearrange("(o p) s -> p o s", p=128))
            ps = pnat()
            for sc in range(4):
                for d in range(ND):
                    nc.tensor.matmul(ps[:, sc * 128:(sc + 1) * 128],
                                     vts[:, d, sc * 128:(sc + 1) * 128],
                                     wv_sb[:, d, :],
                                     start=(d == 0), stop=(d == ND - 1),
                                     skip_group_check=True)
            for sc in range(4):
                c = t * 4 + sc
                nc.scalar.copy(
                    v_sb[:, c, :, 0:64],
                    ps[:, sc * 128:(sc + 1) * 128].rearrange("p (h f) -> p h f", h=2))

        # ---- natural max pass pieces
        mxs_all = {}

        def nat_unit(b, sh, h):
            qsl = slice(b * 128, (b + 1) * 128)
            ssl = slice(sh * 512, (sh + 1) * 512)
            hp = slice(h * 64, (h + 1) * 64)
            psn = pnat()
            nc.tensor.matmul(psn[:], q16[hp, qsl], k16[hp, ssl],
                             start=True, stop=True)
            if (b, h) not in mxs_all:
                mxs_all[(b, h)] = smalls.tile(
                    [128, NSH], f32, tag=f"mx{b % 4}_{h}", name=f"mx{b % 4}_{h}")
            nc.vector.reduce_max(mxs_all[(b, h)][:, sh:sh + 1], psn[:], axis=X)

        def nat_finish(b):
            qsl = slice(b * 128, (b + 1) * 128)
            m2 = smalls.tile([128, 2], fp16, tag="m2", name="m2")
            for h in range(2):
                nc.vector.reduce_max(m2[:, h:h + 1], mxs_all.pop((b, h))[:],
                                     axis=X, negate=True)
            for h in range(2):
                psmt = pnat()
                nc.tensor.matmul(psmt[0:1, 0:128], m2[:, h:h + 1], identp[:],
                                 start=True, stop=True)
                nc.vector.tensor_copy(qaug[h][64:65, qsl], psmt[0:1, 0:128])

        # ---- phase A: q-tile-0 proj first, then kt proj with the natural
        # pass streamed into each tile's DMA window, then v proj, qt tile 1.
        proj_qk_tile(qt, wq_sb, qaug[0], qaug[1], q16, 0)
        for sh in range(NSH):
            proj_qk_tile(kt, wk_sb, kaug[0], kaug[1], k16, sh)
            for b in range(4):
                for h in range(2):
                    nat_unit(b, sh, h)
        for sh in range(NSH):
            proj_v_tile(sh)
        proj_qk_tile(qt, wq_sb, qaug[0], qaug[1], q16, 1)
        for b in range(4):
            nat_finish(b)

        # ---- main loop over q tiles
        wo_queue = []  # deferred W_O block indices (qb values)

        def wo_block(qb, copy_eng=None):
            pso = pfine()
            for n in range(2):
                nc.tensor.matmul(pso[:, n * 512:(n + 1) * 512],
                                 concat[:, qb * 128:(qb + 1) * 128],
                                 wo_sb[:, n * 512:(n + 1) * 512],
                                 start=True, stop=True)
            ot = outp.tile([128, 1024], fp16, tag="ot", name="ot")
            (copy_eng or nc.scalar.copy)(ot[:], pso[:])
            nc.sync.dma_start(out[qb * 128:(qb + 1) * 128, :], ot[:])

        for t in range(NT):
            tsl = slice(t * 512, (t + 1) * 512)
            psAs = [ps_av.tile([65, 512], f32, tag=f"pav{h}", name=f"pav{h}")
                    for h in range(2)]
            es = {}
            for c in range(NCH):
                psf = pfine()
                for h in range(2):
                    nc.tensor.matmul(psf[:, h * 512:(h + 1) * 512],
                                     kaug[h][:, c * 128:(c + 1) * 128],
                                     qaug[h][:, tsl],
                                     start=True, stop=True)
                if c >= 2:
                    eprev = es.pop(c - 2)
                    for h in range(2):
                        nc.tensor.matmul(psAs[h][:], v_sb[:, c - 2, h, :],
                                         eprev[h][:],
                                         start=(c - 2 == 0), stop=False,
                                         skip_group_check=True)
                es[c] = []
                for h in range(2):
                    e = epool.tile([128, 512], fp16, tag=f"e{h}", name=f"e{h}")
                    nc.scalar.activation(e[:], psf[:, h * 512:(h + 1) * 512], Exp)
                    es[c].append(e)
                # W_O of previous tile, spread through this loop
                if c % 8 == 1 and wo_queue:
                    wo_block(wo_queue.pop(0))
                # stream next q-tile's natural pass; finish one step late so
                # the psmt matmul never blocks the in-order PE queue on DVE
                if t + 1 < NT:
                    b, sh = divmod(c, NSH)
                    for h in range(2):
                        nat_unit((t + 1) * 4 + b, sh, h)
                    if c % NSH == 0 and c >= NSH:
                        nat_finish((t + 1) * 4 + c // NSH - 1)
            # tail AV chunks
            for cc in (NCH - 2, NCH - 1):
                eprev = es.pop(cc)
                for h in range(2):
                    nc.tensor.matmul(psAs[h][:], v_sb[:, cc, h, :],
                                     eprev[h][:],
                                     start=False, stop=(cc == NCH - 1),
                                     skip_group_check=True)
            if t + 1 < NT:
                nat_finish((t + 1) * 4 + 3)

            # normalize: concat[h] = att@V * (1/rowsum), transposed layout
            for h in range(2):
                psA = psAs[h]
                sums = smalls.tile([1, 512], f32, tag="sums", name="sums", bufs=1)
                nc.vector.tensor_copy(sums[:], psA[64:65, :])
                recf = smalls.tile([1, 512], f32, tag="recf", name="recf", bufs=1)
                nc.vector.reciprocal_approx_fast(recf[:], sums[:])
                rec = smalls.tile([1, 512], f32r, tag="rec", name="rec", bufs=1)
                nc.vector.tensor_copy(rec[:], recf[:])
                psr = pfine()
                nc.tensor.matmul(psr[0:64, 0:512], ones64[:], rec[:],
                                 start=True, stop=True)
                reps = smalls.tile([64, 512], f32, tag="reps", name="reps", bufs=1)
                nc.scalar.copy(reps[:], psr[0:64, 0:512])
                nc.vector.tensor_mul(concat[h * 64:(h + 1) * 64, tsl],
                                     psA[0:64, :], reps[:])

            # qt proj for tile t+2 (needed by nat pass streaming during t+1)
            if t + 2 < NT:
                proj_qk_tile(qt, wq_sb, qaug[0], qaug[1], q16, t + 2)

            wo_queue.extend(t * 4 + b for b in range(4))
            if t == NT - 1:
                for i in range(len(wo_queue)):
                    wo_block(wo_queue.pop(0))

        if dbg is not None:
            nc.sync.dma_start(dbg["d_qaug0"][:], qaug[0][:].bitcast(f32))
            nc.sync.dma_start(dbg["d_kaug0"][:], kaug[0][:].bitcast(f32))
            nc.sync.dma_start(dbg["d_v"][:], v_sb[:].rearrange("p a b c -> p (a b c)"))
            nc.sync.dma_start(dbg["d_concat"][:], concat[:].bitcast(f32))

    nc.compile()
    return nc


def _prep_inputs(Q, K, V, W_Q, W_K, W_V, W_O):
    Q = np.ascontiguousarray(np.asarray(Q, dtype=np.float32))
    K = np.ascontiguousarray(np.asarray(K, dtype=np.float32))
    V = np.ascontiguousarray(np.asarray(V, dtype=np.float32))
    W_Q = np.asarray(W_Q, dtype=np.float32)
    W_K = np.asarray(W_K, dtype=np.float32)
    W_V = np.asarray(W_V, dtype=np.float32)
    W_O = np.asarray(W_O, dtype=np.float32)

    QT = np.ascontiguousarray(Q.T)
    KT = np.ascontiguousarray(K.T)
    VT = np.ascontiguousarray(V.T.astype(np.float16))
    scale = np.float32(0.125)  # 1/sqrt(64), exact power of two

    in_maps = []
    for c in range(NCORES):
        hA, hB = 2 * c, 2 * c + 1
        in_maps.append({
            "qt": QT,
            "kt": KT,
            "vt": VT,
            "wq": np.ascontiguousarray(np.concatenate([W_Q[hA], W_Q[hB]], axis=1)),
            "wk": np.ascontiguousarray(
                np.concatenate([W_K[hA] * scale, W_K[hB] * scale], axis=1)),
            "wv": np.ascontiguousarray(
                np.concatenate([W_V[hA], W_V[hB]], axis=1).astype(np.float16)),
            "wo": np.ascontiguousarray(W_O[c * 128:(c + 1) * 128, :]),
        })
    return in_maps


def kernel(Q, K, V, W_Q, W_K, W_V, W_O):
    global LAST_RESULT
    from concourse.bass_utils import run_bass_kernel_spmd

    S = np.asarray(Q).shape[0]
    nc = _cache.get(S)
    if nc is None:
        nc = _build(S)
        _cache[S] = nc

    in_maps = _prep_inputs(Q, K, V, W_Q, W_K, W_V, W_O)
    res = run_bass_kernel_spmd(nc, in_maps, list(range(NCORES)))
    LAST_RESULT = res
    parts = np.stack(
        [res.results[i]["out"].astype(np.float32) for i in range(NCORES)])
    return parts.sum(axis=0, dtype=np.float32)


# revision 40
# speedup vs baseline: 1.2111x; 1.0614x over previous
"""Trainium2 Bass kernel: 16-head attention (SEQ=4096, D_MODEL=1024, D_K=64).

Sharding: tensor-parallel over heads. 2 heads per core x 8 cores.
W_O is row-sharded; each core returns a partial [S, D] output projection,
summed on the host (the all-reduce of the output projection).

Per-core dataflow (score matmuls fp32r = FP22-truncated full-rate):
  qaug/kaug [65, S] per head via projections on transposed inputs; fp16
    copies q16/k16 feed the natural max pass.
  v natural [S, 64+1] fp16 per head (ones column yields softmax row sums).
  Natural pass (scores [q,s], fp16): DVE reduce_max per [128,512] unit into
    mxs, final reduce -> m2, transposed via a tiny -I matmul into qaug row 64.
    Streams inside the previous q-tile's main loop on a dedicated 2-buffer
    PSUM pool so reduce reads never stall the PE queue.
  Fine pass (transposed scores [s,q]): K=65 f32r matmul gives
    scores^T - max(q) on a 2-buffer [128,1024] PSUM pool; per-head ScalarE
    exp -> E fp16; [v|1] @ E accumulates att@V + row sums with a 2-chunk
    skew so PE never waits on the exp.
  Normalize via reciprocal + ones-replication matmul; W_O blocks of tile
    t-1 interleave into tile t's loop (c%8==1) on the fine pool.
"""

import os
import sys

import numpy as np

for _p in (
    "/root/.axon_site",
    "/root/.axon_site/_ro/trn_rl_repo",
    "/root/.axon_site/_ro/pypackages",
    "/opt/trn_rl_repo",
    "/opt/pypackages",
):
    if os.path.isdir(_p) and _p not in sys.path:
        sys.path.append(_p)

D = 1024
NHEADS = 16
DK = 64
NCORES = 8
S_FULL = 4096

_cache = {}
LAST_RESULT = None  # BassKernelResults of the most recent run (for test harness)


def _build(S):
    import concourse.bass as bass  # noqa: F401
    import concourse.tile as tile
    from concourse import bacc, mybir
    from concourse.masks import make_identity
    from contextlib import ExitStack

    f32 = mybir.dt.float32
    f32r = mybir.dt.float32r
    fp16 = mybir.dt.float16
    X = mybir.AxisListType.X
    Exp = mybir.ActivationFunctionType.Exp

    NT = S // 512   # 512-wide q tiles
    NCH = S // 128  # 128-wide s chunks
    ND = D // 128   # contraction chunks
    NSH = S // 512  # 512-wide s tiles

    dbg_on = bool(int(os.environ.get("KDBG", "0")))
    nc = bacc.Bacc(
        "TRN2",
        target_bir_lowering=False,
        debug=False,
        num_devices=NCORES,
    )
    qt = nc.dram_tensor("qt", [D, S], f32r, kind="ExternalInput")
    kt = nc.dram_tensor("kt", [D, S], f32r, kind="ExternalInput")
    vt = nc.dram_tensor("vt", [D, S], fp16, kind="ExternalInput")
    wq = nc.dram_tensor("wq", [D, 128], f32r, kind="ExternalInput")
    wk = nc.dram_tensor("wk", [D, 128], f32r, kind="ExternalInput")
    wv = nc.dram_tensor("wv", [D, 128], fp16, kind="ExternalInput")
    wo = nc.dram_tensor("wo", [128, D], f32r, kind="ExternalInput")
    out = nc.dram_tensor("out", [S, D], fp16, kind="ExternalOutput")
    dbg = None
    if dbg_on:
        dbg = {
            "d_qaug0": nc.dram_tensor("d_qaug0", [65, S], f32, kind="ExternalOutput"),
            "d_kaug0": nc.dram_tensor("d_kaug0", [65, S], f32, kind="ExternalOutput"),
            "d_v": nc.dram_tensor("d_v", [128, (S // 128) * 2 * 65], fp16,
                                  kind="ExternalOutput"),
            "d_concat": nc.dram_tensor("d_concat", [128, S], f32,
                                       kind="ExternalOutput"),
        }

    with tile.TileContext(nc) as tc, ExitStack() as ctx:
        consts = ctx.enter_context(tc.tile_pool(name="consts", bufs=1))
        big = ctx.enter_context(tc.tile_pool(name="big", bufs=1))
        ldpool = ctx.enter_context(tc.tile_pool(name="ld", bufs=2))
        epool = ctx.enter_context(tc.tile_pool(name="e", bufs=3))
        smalls = ctx.enter_context(tc.tile_pool(name="smalls", bufs=2))
        outp = ctx.enter_context(tc.tile_pool(name="outp", bufs=2))
        ps_f = ctx.enter_context(tc.tile_pool(name="ps_f", bufs=2, space="PSUM"))
        ps_n = ctx.enter_context(tc.tile_pool(name="ps_n", bufs=2, space="PSUM"))
        ps_av = ctx.enter_context(tc.tile_pool(name="ps_av", bufs=1, space="PSUM"))

        def pfine():
            return ps_f.tile([128, 1024], f32, tag="pf", name="pf")

        def pnat():
            return ps_n.tile([128, 512], f32, tag="pn", name="pn")

        # constants
        ident_f = consts.tile([128, 128], f32)
        make_identity(nc, ident_f)
        identn = consts.tile([128, 128], f32r)  # -I, rounded for fp32r matmul
        nc.vector.tensor_scalar_mul(identn[:], ident_f[:], -1.0)
        ones64 = consts.tile([1, 64], f32r)
        nc.vector.memset(ones64[:].bitcast(f32), 1.0)

        # weights
        wq_sb = consts.tile([128, ND, 128], f32r)
        nc.sync.dma_start(wq_sb[:], wq.rearrange("(o p) f -> p o f", p=128))
        wk_sb = consts.tile([128, ND, 128], f32r)
        nc.sync.dma_start(wk_sb[:], wk.rearrange("(o p) f -> p o f", p=128))
        wv_sb = consts.tile([128, ND, 128], fp16)
        nc.sync.dma_start(wv_sb[:], wv.rearrange("(o p) f -> p o f", p=128))
        wo_sb = consts.tile([128, D], f32r)
        nc.sync.dma_start(wo_sb[:], wo[:])

        # big SBUF tensors
        qaug = [big.tile([65, S], f32r, tag=f"qaug{h}", name=f"qaug{h}") for h in range(2)]
        kaug = [big.tile([65, S], f32r, tag=f"kaug{h}", name=f"kaug{h}") for h in range(2)]
        v_sb = big.tile([128, NCH, 2, 65], fp16, tag="v", name="v_sb")
        q16 = big.tile([128, S], fp16, tag="q16", name="q16")
        k16 = big.tile([128, S], fp16, tag="k16", name="k16")
        concat = big.tile([128, S], f32r, tag="concat", name="concat")
        for h in range(2):
            nc.vector.memset(kaug[h][64:65, :].bitcast(f32), 1.0)
        nc.gpsimd.memset(v_sb[:, :, :, 64:65], 1.0)

        # ---- per-tile projection helpers
        def proj_qk_tile(src, wsb, dstA, dstB, dst16, t):
            lt = ldpool.tile([128, ND, 512], f32r, tag="ld", name="ld", bufs=2)
            nc.sync.dma_start(
                lt[:], src[:, t * 512:(t + 1) * 512].rearrange("(o p) s -> p o s", p=128))
            ps = pnat()
            for d in range(ND):
                nc.tensor.matmul(ps[:], wsb[:, d, :], lt[:, d, :],
                                 start=(d == 0), stop=(d == ND - 1))
            nc.scalar.copy(dstA[0:64, t * 512:(t + 1) * 512], ps[0:64, :])
            nc.scalar.copy(dstB[0:64, t * 512:(t + 1) * 512], ps[64:128, :])
            nc.scalar.copy(dst16[:, t * 512:(t + 1) * 512], ps[:])

        def proj_v_tile(t):
            # sc-major: one accumulation group at a time per PSUM bank — a
            # group's start=True clears has_written for the whole bank, so
            # groups must not interleave within a bank.
            vts = ldpool.tile([128, ND, 512], fp16, tag="vld", name="vld", bufs=2)
            nc.sync.dma_start(
                vts[:], vt[:, t * 512:(t + 1) * 512].rearrange("(o p) s -> p o s", p=128))
            ps = pnat()
            for sc in range(4):
                for d in range(ND):
                    nc.tensor.matmul(ps[:, sc * 128:(sc + 1) * 128],
                                     vts[:, d, sc * 128:(sc + 1) * 128],
                                     wv_sb[:, d, :],
                                     start=(d == 0), stop=(d == ND - 1),
                                     skip_group_check=True)
            for sc in range(4):
                c = t * 4 + sc
                nc.scalar.copy(
                    v_sb[:, c, :, 0:64],
                    ps[:, sc * 128:(sc + 1) * 128].rearrange("p (h f) -> p h f", h=2))

        # ---- natural max pass pieces
        mxs_all = {}

        def nat_unit(b, sh, h):
            qsl = slice(b * 128, (b + 1) * 128)
            ssl = slice(sh * 512, (sh + 1) * 512)
            hp = slice(h * 64, (h + 1) * 64)
            psn = pnat()
            nc.tensor.matmul(psn[:], q16[hp, qsl], k16[hp, ssl],
                             start=True, stop=True)
            if (b, h) not in mxs_all:
                mxs_all[(b, h)] = smalls.tile(
                    [128, NSH], f32, tag=f"mx{b % 4}_{h}", name=f"mx{b % 4}_{h}")
            nc.vector.reduce_max(mxs_all[(b, h)][:, sh:sh + 1], psn[:], axis=X)

        def nat_finish(b):
            qsl = slice(b * 128, (b + 1) * 128)
            m2 = smalls.tile([128, 2], f32r, tag="m2", name="m2")
            for h in range(2):
                nc.vector.reduce_max(m2[:, h:h + 1], mxs_all.pop((b, h))[:], axis=X)
            for h in range(2):
                psmt = pnat()
                nc.tensor.matmul(psmt[0:1, 0:128], m2[:, h:h + 1], identn[:],
                                 start=True, stop=True)
                nc.vector.tensor_copy(qaug[h][64:65, qsl], psmt[0:1, 0:128])

        # ---- phase A: q-tile-0 proj first, then kt proj with the natural
        # pass streamed into each tile's DMA window, then v proj, qt tile 1.
        proj_qk_tile(qt, wq_sb, qaug[0], qaug[1], q16, 0)
        for sh in range(NSH):
            proj_qk_tile(kt, wk_sb, kaug[0], kaug[1], k16, sh)
            for b in range(4):
                for h in range(2):
                    nat_unit(b, sh, h)
        for sh in range(NSH):
            proj_v_tile(sh)
        proj_qk_tile(qt, wq_sb, qaug[0], qaug[1], q16, 1)
        for b in range(4):
            nat_finish(b)

        # ---- main loop over q tiles
        wo_queue = []  # deferred W_O block indices (qb values)

        def wo_block(qb, copy_eng=None):
            pso = pfine()
            for n in range(2):
                nc.tensor.matmul(pso[:, n * 512:(n + 1) * 512],
                                 concat[:, qb * 128:(qb + 1) * 128],
                                 wo_sb[:, n * 512:(n + 1) * 512],
                                 start=True, stop=True)
            ot = outp.tile([128, 1024], fp16, tag="ot", name="ot")
            (copy_eng or nc.scalar.copy)(ot[:], pso[:])
            nc.sync.dma_start(out[qb * 128:(qb + 1) * 128, :], ot[:])

        for t in range(NT):
            tsl = slice(t * 512, (t + 1) * 512)
            psAs = [ps_av.tile([65, 512], f32, tag=f"pav{h}", name=f"pav{h}")
                    for h in range(2)]
            es = {}
            for c in range(NCH):
                psf = pfine()
                for h in range(2):
                    nc.tensor.matmul(psf[:, h * 512:(h + 1) * 512],
                                     kaug[h][:, c * 128:(c + 1) * 128],
                                     qaug[h][:, tsl],
                                     start=True, stop=True)
                if c >= 2:
                    eprev = es.pop(c - 2)
                    for h in range(2):
                        nc.tensor.matmul(psAs[h][:], v_sb[:, c - 2, h, :],
                                         eprev[h][:],
                                         start=(c - 2 == 0), stop=False,
                                         skip_group_check=True)
                es[c] = []
                for h in range(2):
                    e = epool.tile([128, 512], fp16, tag=f"e{h}", name=f"e{h}")
                    nc.scalar.activation(e[:], psf[:, h * 512:(h + 1) * 512], Exp)
                    es[c].append(e)
                # W_O of previous tile, spread through this loop
                if c % 8 == 1 and wo_queue:
                    wo_block(wo_queue.pop(0))
                # stream next q-tile's natural pass; finish one step late so
                # the psmt matmul never blocks the in-order PE queue on DVE
                if t + 1 < NT:
                    b, sh = divmod(c, NSH)
                    for h in range(2):
                        nat_unit((t + 1) * 4 + b, sh, h)
                    if c % NSH == 0 and c >= NSH:
                        nat_finish((t + 1) * 4 + c // NSH - 1)
            # tail AV chunks
            for cc in (NCH - 2, NCH - 1):
                eprev = es.pop(cc)
                for h in range(2):
                    nc.tensor.matmul(psAs[h][:], v_sb[:, cc, h, :],
                                     eprev[h][:],
                                     start=False, stop=(cc == NCH - 1),
                                     skip_group_check=True)
            if t + 1 < NT:
                nat_finish((t + 1) * 4 + 3)

            # normalize: concat[h] = att@V * (1/rowsum), transposed layout
            for h in range(2):
                psA = psAs[h]
                sums = smalls.tile([1, 512], f32, tag="sums", name="sums", bufs=1)
                nc.vector.tensor_copy(sums[:], psA[64:65, :])
                recf = smalls.tile([1, 512], f32, tag="recf", name="recf", bufs=1)
                nc.vector.reciprocal_approx_fast(recf[:], sums[:])
                rec = smalls.tile([1, 512], f32r, tag="rec", name="rec", bufs=1)
                nc.vector.tensor_copy(rec[:], recf[:])
                psr = pfine()
                nc.tensor.matmul(psr[0:64, 0:512], ones64[:], rec[:],
                                 start=True, stop=True)
                reps = smalls.tile([64, 512], f32, tag="reps", name="reps", bufs=1)
                nc.scalar.copy(reps[:], psr[0:64, 0:512])
                nc.vector.tensor_mul(concat[h * 64:(h + 1) * 64, tsl],
                                     psA[0:64, :], reps[:])

            # qt proj for tile t+2 (needed by nat pass streaming during t+1)
            if t + 2 < NT:
                proj_qk_tile(qt, wq_sb, qaug[0], qaug[1], q16, t + 2)

            wo_queue.extend(t * 4 + b for b in range(4))
            if t == NT - 1:
                for i in range(len(wo_queue)):
                    wo_block(wo_queue.pop(0))

        if dbg is not None:
            nc.sync.dma_start(dbg["d_qaug0"][:], qaug[0][:].bitcast(f32))
            nc.sync.dma_start(dbg["d_kaug0"][:], kaug[0][:].bitcast(f32))
            nc.sync.dma_start(dbg["d_v"][:], v_sb[:].rearrange("p a b c -> p (a b c)"))
            nc.sync.dma_start(dbg["d_concat"][:], concat[:].bitcast(f32))

    nc.compile()
    return nc


def _prep_inputs(Q, K, V, W_Q, W_K, W_V, W_O):
    Q = np.ascontiguousarray(np.asarray(Q, dtype=np.float32))
    K = np.ascontiguousarray(np.asarray(K, dtype=np.float32))
    V = np.ascontiguousarray(np.asarray(V, dtype=np.float32))
    W_Q = np.asarray(W_Q, dtype=np.float32)
    W_K = np.asarray(W_K, dtype=np.float32)
    W_V = np.asarray(W_V, dtype=np.float32)
    W_O = np.asarray(W_O, dtype=np.float32)

    QT = np.ascontiguousarray(Q.T)
    KT = np.ascontiguousarray(K.T)
    VT = np.ascontiguousarray(V.T.astype(np.float16))
    scale = np.float32(0.125)  # 1/sqrt(64), exact power of two

    in_maps = []
    for c in range(NCORES):
        hA, hB = 2 * c, 2 * c + 1
        in_maps.append({
            "qt": QT,
            "kt": KT,
            "vt": VT,
            "wq": np.ascontiguousarray(np.concatenate([W_Q[hA], W_Q[hB]], axis=1)),
            "wk": np.ascontiguousarray(
                np.concatenate([W_K[hA] * scale, W_K[hB] * scale], axis=1)),
            "wv": np.ascontiguousarray(
                np.concatenate([W_V[hA], W_V[hB]], axis=1).astype(np.float16)),
            "wo": np.ascontiguousarray(W_O[c * 128:(c + 1) * 128, :]),
        })
    return in_maps


def kernel(Q, K, V, W_Q, W_K, W_V, W_O):
    global LAST_RESULT
    from concourse.bass_utils import run_bass_kernel_spmd

    S = np.asarray(Q).shape[0]
    nc = _cache.get(S)
    if nc is None:
        nc = _build(S)
        _cache[S] = nc

    in_maps = _prep_inputs(Q, K, V, W_Q, W_K, W_V, W_O)
    res = run_bass_kernel_spmd(nc, in_maps, list(range(NCORES)))
    LAST_RESULT = res
    parts = np.stack(
        [res.results[i]["out"].astype(np.float32) for i in range(NCORES)])
    return parts.sum(axis=0, dtype=np.float32)
